# revision 1
# baseline (speedup 1.0000x reference)
"""Causal self-attention (B=4, S=2048, D=1024, H=16) on 8 Trainium2 NeuronCores.

Sharding: 8 cores = 4 batches x 2 head-groups (8 heads each).
Per core: QKV projections, flash-style causal attention with scores computed
transposed ([k, q] layout), exp on ScalarE (no max subtraction -- scores are
O(1) here), softmax denominator via an appended ones-column in the attn@V
matmul, out-projection against a W_O column slice.  The inter-core
"all-reduce" for the out-projection (row-parallel W_O) is a host-side sum of
the two head-group partials per batch.

All matmuls are bf16 (fp32 PSUM accumulation).  Contractions of K=128 are
split into two K=64 halves on alternating PE row-groups so LDWEIGHTS of one
half overlaps the other half's MATMUL and the two halves execute concurrently
on disjoint quadrant rows.  The two heads of a 128-row Q^T/K^T chunk are
interleaved for the same reason.
"""

import os
import sys

for _p in ("/opt/trn_rl_repo",):
    if _p not in sys.path and os.path.isdir(_p):
        sys.path.insert(0, _p)

import numpy as np

B, S, D, H, DK = 4, 2048, 1024, 16, 64
N_CORES = 8
EC = 512          # e-dims (= head-dim columns) per core: 8 heads x 64
N_D = D // 128    # 8 contraction chunks for projections
N_SC = S // 128   # 16 key chunks
N_QB = S // 512   # 4 query blocks

_CACHE = {}


def _build():
    import concourse.mybir as mybir
    import concourse.tile as tile
    from concourse import bacc
    from contextlib import ExitStack

    fp32 = mybir.dt.float32
    bf16 = mybir.dt.bfloat16
    AF = mybir.ActivationFunctionType
    Alu = mybir.AluOpType

    nc = bacc.Bacc(trn_type="TRN2", target_bir_lowering=False, debug=False)

    xt_d = nc.dram_tensor("xt", [D, S], bf16, kind="ExternalInput")
    wq_d = nc.dram_tensor("wqt", [D, EC], bf16, kind="ExternalInput")
    wk_d = nc.dram_tensor("wkt", [D, EC], bf16, kind="ExternalInput")
    wv_d = nc.dram_tensor("wvt", [D, EC], bf16, kind="ExternalInput")
    wo_d = nc.dram_tensor("wot", [EC, D], bf16, kind="ExternalInput")
    yt_d = nc.dram_tensor("yt", [D, S], fp32, kind="ExternalOutput")

    with tile.TileContext(nc) as tc, ExitStack() as ctx:
        # ---- persistent results of phase 1 ------------------------------
        proj_out_pool = ctx.enter_context(tc.tile_pool(name="projout", bufs=1))
        qt_sb = [proj_out_pool.tile([128, S], bf16, name=f"qt{ec}", tag=f"qt{ec}") for ec in range(4)]
        kt_sb = [proj_out_pool.tile([128, S], bf16, name=f"kt{ec}", tag=f"kt{ec}") for ec in range(4)]
        # v_sb[sc]: per head h a 128-col stationary block:
        #   even h: [V(64) | ones(col 64) | unused(63)] -> psum rows 0..64
        #   odd  h: [zeros(0:63), ones at col 32 | V(64) at 64:128]
        #           -> psum row 32 = n, rows 64..127 = out
        v_sb = [proj_out_pool.tile([128, 8, 128], bf16, name=f"v{sc}", tag=f"v{sc}") for sc in range(N_SC)]

        # ---- input tiles (all bf16) -------------------------------------
        xw_pool = ctx.enter_context(tc.tile_pool(name="xw", bufs=1))
        xt_sb, wq_sb, wk_sb, wv_sb = [], [], [], []
        for d in range(N_D):
            t = xw_pool.tile([128, S], bf16, name=f"x{d}", tag=f"x{d}")
            nc.sync.dma_start(t[:], xt_d.ap()[128 * d:128 * (d + 1), :])
            xt_sb.append(t)
            for lst, dram, nm in ((wq_sb, wq_d, "q"), (wk_sb, wk_d, "k"), (wv_sb, wv_d, "v")):
                t = xw_pool.tile([128, EC], bf16, name=f"w{nm}{d}", tag=f"w{nm}{d}")
                nc.gpsimd.dma_start(t[:], dram.ap()[128 * d:128 * (d + 1), :])
                lst.append(t)
        wo_sb = []
        for cc in range(4):
            t = xw_pool.tile([128, D], bf16, name=f"wo{cc}", tag=f"wo{cc}")
            nc.gpsimd.dma_start(t[:], wo_d.ap()[128 * cc:128 * (cc + 1), :])
            wo_sb.append(t)

        # ---- constant masks for the 4 diagonal positions (emitted after the
        # input DMAs so the gpsimd queue issues those first) ---------------
        const_pool = ctx.enter_context(tc.tile_pool(name="const", bufs=1))
        masks = []
        for j in range(4):
            m = const_pool.tile([128, 512], bf16, name=f"mask{j}", tag=f"mask{j}")
            nc.gpsimd.memset(m[:], 1.0)
            # keep 1.0 where q_rel >= p + 128*j  (q >= k), else 0
            nc.gpsimd.affine_select(
                out=m[:], in_=m[:], compare_op=Alu.is_ge, fill=0.0,
                base=-128 * j, pattern=[[1, 512]], channel_multiplier=-1,
            )
            masks.append(m)

        dram_pool = ctx.enter_context(tc.tile_pool(name="drs", bufs=4, space="DRAM"))
        ps_score = ctx.enter_context(tc.tile_pool(name="psscore", bufs=1, space="PSUM"))
        ps_av = ctx.enter_context(tc.tile_pool(name="psav", bufs=1, space="PSUM"))
        ps_proj = tc.alloc_tile_pool(name="psproj", bufs=2, space="PSUM")
        attn_pool = ctx.enter_context(tc.tile_pool(name="attn", bufs=3))
        rb_pool = ctx.enter_context(tc.tile_pool(name="rb", bufs=3))
        ou_pool = ctx.enter_context(tc.tile_pool(name="ou", bufs=3))
        outn_pool = ctx.enter_context(tc.tile_pool(name="outn", bufs=4))
        y_pool = ctx.enter_context(tc.tile_pool(name="ysb", bufs=3))

        def emit_qk_proj(ec):
            for sb_ in range(4):
                for w_sb, out_sb, nm in ((wq_sb, qt_sb, "q"), (wk_sb, kt_sb, "k")):
                    ps = ps_proj.tile([128, 512], fp32, name="pp", tag="pp")
                    for d in range(N_D):
                        nc.tensor.matmul(
                            ps[:],
                            w_sb[d][:, 128 * ec:128 * (ec + 1)],
                            xt_sb[d][:, 512 * sb_:512 * (sb_ + 1)],
                            start=(d == 0), stop=(d == N_D - 1),
                        )
                    nc.vector.tensor_copy(out_sb[ec][:, 512 * sb_:512 * (sb_ + 1)], ps[:])

        def emit_v_proj(sc):
            ps = ps_proj.tile([128, 512], fp32, name="pv", tag="pp")
            for d in range(N_D):
                nc.tensor.matmul(
                    ps[:],
                    xt_sb[d][:, 128 * sc:128 * (sc + 1)],
                    wv_sb[d][:],
                    start=(d == 0), stop=(d == N_D - 1),
                )
            vt = v_sb[sc]
            for h in range(8):
                if h % 2 == 0:
                    nc.vector.tensor_copy(vt[:, h, 0:64], ps[:, 64 * h:64 * h + 64])
                    nc.gpsimd.memset(vt[:, h, 64:65], 1.0)
                else:
                    nc.gpsimd.memset(vt[:, h, 0:63], 0.0)
                    nc.gpsimd.memset(vt[:, h, 32:33], 1.0)
                    nc.vector.tensor_copy(vt[:, h, 64:128], ps[:, 64 * h:64 * h + 64])

        def scores_mm(ps_s, kt, qt, prow, kc, j, qb):
            nc.tensor.matmul(
                ps_s[:, 512 * j:512 * (j + 1)],
                kt[prow:prow + 64, 128 * kc:128 * (kc + 1)],
                qt[prow:prow + 64, 512 * qb:512 * (qb + 1)],
                start=True, stop=True,
            )

        def av_mms(ps_o, h, at, kt_i, nkc):
            m_sz = 65 if h % 2 == 0 else 128
            for j in (0, 1):
                kc = 2 * kt_i + j
                nc.tensor.matmul(
                    ps_o[0:m_sz, :],
                    v_sb[kc][:, h, 0:m_sz],
                    at[:, 512 * j:512 * (j + 1)],
                    start=(kc == 0), stop=(kc == nkc - 1),
                )

        def emit_attn(qb, hp, outn):
            hA, hB = 2 * hp, 2 * hp + 1
            qt, kt = qt_sb[hp], kt_sb[hp]
            nkc = 4 * qb + 4
            ps_oA = ps_av.tile([128, 512], fp32, name="poA", tag="poA")
            ps_oB = ps_av.tile([128, 512], fp32, name="poB", tag="poB")
            pend = None
            for kt_i in range(nkc // 2):
                ps_sA = ps_score.tile([128, 1024], fp32, name="psA", tag="psA")
                ps_sB = ps_score.tile([128, 1024], fp32, name="psB", tag="psB")
                for j in (0, 1):
                    kc = 2 * kt_i + j
                    scores_mm(ps_sA, kt, qt, 0, kc, j, qb)
                    scores_mm(ps_sB, kt, qt, 64, kc, j, qb)
                atA = attn_pool.tile([128, 1024], bf16, name="atA", tag="atA")
                atB = attn_pool.tile([128, 1024], bf16, name="atB", tag="atB")
                nc.scalar.activation(atA[:], ps_sA[:], AF.Exp, scale=0.125)
                nc.scalar.activation(atB[:], ps_sB[:], AF.Exp, scale=0.125)
                for at in (atA, atB):
                    for j in (0, 1):
                        jj = 2 * kt_i + j - 4 * qb
                        if jj >= 0:
                            nc.vector.tensor_mul(
                                at[:, 512 * j:512 * (j + 1)],
                                at[:, 512 * j:512 * (j + 1)], masks[jj][:])
                if pend is not None:
                    av_mms(ps_oA, hA, pend[0], pend[2], nkc)
                    av_mms(ps_oB, hB, pend[1], pend[2], nkc)
                pend = (atA, atB, kt_i)
            av_mms(ps_oA, hA, pend[0], pend[2], nkc)
            av_mms(ps_oB, hB, pend[1], pend[2], nkc)

            # normalization: copy out + n rows off PSUM (frees banks),
            # broadcast both n rows into one base-0 tile via DRAM, one
            # full-tile fast reciprocal (base-0 only!), gpsimd muls.
            ou = ou_pool.tile([128, 512], fp32, name="ou", tag="ou")
            rbn = rb_pool.tile([128, 512], fp32, name="rbn", tag="rbn")
            rbi = rb_pool.tile([128, 512], fp32, name="rbi", tag="rbi")
            rbb = rb_pool.tile([128, 512], fp32, name="rbb", tag="rbb")
            nc.vector.tensor_copy(ou[0:64, :], ps_oA[0:64, :])
            nc.vector.tensor_copy(rbn[64:65, :], ps_oA[64:65, :])
            nc.vector.tensor_copy(ou[64:128, :], ps_oB[64:128, :])
            nc.vector.tensor_copy(rbn[32:33, :], ps_oB[32:33, :])
            rdA = dram_pool.tile([1, 512], fp32, name="rdA", tag="rdA")
            rdB = dram_pool.tile([1, 512], fp32, name="rdB", tag="rdB")
            nc.sync.dma_start(rdA[:], rbn[64:65, :])
            nc.sync.dma_start(rbb[0:64, :], rdA[0:1, :].to_broadcast((64, 512)))
            nc.sync.dma_start(rdB[:], rbn[32:33, :])
            nc.sync.dma_start(rbb[64:128, :], rdB[0:1, :].to_broadcast((64, 512)))
            nc.vector.reciprocal_approx_fast(out=rbi[:, :], in_=rbb[:, :])
            mul_eng = nc.vector if hp == 3 else nc.gpsimd
            mul_eng.tensor_mul(outn[hp][0:64, :], ou[0:64, :], rbi[0:64, :])
            mul_eng.tensor_mul(outn[hp][64:128, :], ou[64:128, :], rbi[64:128, :])

        def emit_outproj(qb, outn):
            for dc in range(8):
                ps = ps_y.tile([128, 512], fp32, name="py", tag="py")
                for hp in range(4):
                    nc.tensor.matmul(
                        ps[:],
                        wo_sb[hp][:, 128 * dc:128 * (dc + 1)],
                        outn[hp][:],
                        start=(hp == 0), stop=(hp == 3),
                    )
                ysb = y_pool.tile([128, 512], fp32, name="y", tag="y")
                nc.vector.tensor_copy(ysb[:], ps[:])
                nc.sync.dma_start(
                    yt_d.ap()[128 * dc:128 * (dc + 1), 512 * qb:512 * (qb + 1)],
                    ysb[:])

        # ---- interleaved emission ---------------------------------------
        # attn(qb, hp) becomes emittable after proj block k = max(hp, qb)
        outn_all = {qb: [outn_pool.tile([128, 512], bf16, name=f"on{qb}{hp}", tag=f"on{hp}")
                         for hp in range(4)] for qb in range(N_QB)}
        emitted = set()
        for k in range(4):
            emit_qk_proj(k)
            for sc in range(4 * k, 4 * k + 4):
                emit_v_proj(sc)
            for qb in range(N_QB):
                for hp in range(4):
                    if max(hp, qb) == k and (k < 3):
                        emit_attn(qb, hp, outn_all[qb])
                        emitted.add((qb, hp))
        ps_proj.release()
        ps_y = ctx.enter_context(tc.tile_pool(name="psy", bufs=2, space="PSUM"))
        for qb in range(3):
            emit_attn(qb, 3, outn_all[qb])
            emit_attn(3, qb, outn_all[3])
            emit_outproj(qb, outn_all[qb])
        emit_attn(3, 3, outn_all[3])
        emit_outproj(3, outn_all[3])

    nc.compile()
    return nc


def _get_nc():
    if "nc" not in _CACHE:
        _CACHE["nc"] = _build()
    return _CACHE["nc"]


def _run(in_maps, trace=False, **kw):
    from concourse.bass_utils import run_bass_kernel_spmd
    nc = _get_nc()
    return run_bass_kernel_spmd(nc, in_maps, core_ids=list(range(N_CORES)),
                                trace=trace, **kw)


def _prep_inputs(x, W_Q, W_K, W_V, W_O):
    import ml_dtypes
    bf = ml_dtypes.bfloat16
    x = np.asarray(x, dtype=np.float32)
    W_Q = np.asarray(W_Q, dtype=np.float32)
    W_K = np.asarray(W_K, dtype=np.float32)
    W_V = np.asarray(W_V, dtype=np.float32)
    W_O = np.asarray(W_O, dtype=np.float32)
    in_maps = []
    for c in range(N_CORES):
        b, hg = divmod(c, 2)
        es = EC * hg
        in_maps.append({
            "xt": np.ascontiguousarray(x[b].T).astype(bf),
            "wqt": np.ascontiguousarray(W_Q[es:es + EC, :].T).astype(bf),
            "wkt": np.ascontiguousarray(W_K[es:es + EC, :].T).astype(bf),
            "wvt": np.ascontiguousarray(W_V[es:es + EC, :].T).astype(bf),
            "wot": np.ascontiguousarray(W_O[:, es:es + EC].T).astype(bf),
        })
    return in_maps


def _gather(results):
    y = np.empty((B, S, D), dtype=np.float32)
    for b in range(B):
        yt = results[2 * b]["yt"].astype(np.float32) + results[2 * b + 1]["yt"].astype(np.float32)
        y[b] = yt.T
    return y


def kernel(x, W_Q, W_K, W_V, W_O):
    in_maps = _prep_inputs(x, W_Q, W_K, W_V, W_O)
    res = _run(in_maps, trace=False)
    return _gather(res.results)

